# revision 4
# baseline (speedup 1.0000x reference)
"""GAT attention layer (N=8192, F_in=512, F_out=128) on 8 TRN2 NeuronCores.

Row-sharded: core r owns output rows [1024*r, 1024*(r+1)).

Math notes:
  h  = input @ W                          [N, 128]
  s1 = h @ a1 = input @ (W @ a1),  s2 = h @ a2 = input @ (W @ a2)
  e[i,j]   = leakyrelu(s1[i] + s2[j], 0.2)
  att      = softmax(where(adj>0, e, -big), axis=1)
  out      = elu(att @ h)

  exp(leakyrelu(t)) == max(exp(t), exp(0.2*t))     (monotonicity, both sides
  agree at t=0), so no LeakyRelu activation pass is needed, and since
  adj in {0,1}: where-mask+softmax == (adj * exp(e)) / rowsum(adj * exp(e)).
  Logits |s1+s2| <~ 20 so exp() is safe in fp32 without max-subtraction.

Device layout per core (j = column/source node, i = local row/dest node):
  - P tiles are built TRANSPOSED [j_part=128, i_free=1024] so the final
    matmul contracts j on partitions: out[i, f] = sum_j P[j,i] * hb[j, f].
  - hb rhs = [h | 1] bf16: the ones column makes the same matmul emit the
    softmax row-sum into psum column 128.
"""

import os
import sys

for _p in ("/opt/trn_rl_repo", "/root/.axon_site/_ro/trn_rl_repo"):
    if os.path.isdir(_p) and _p not in sys.path:
        sys.path.insert(0, _p)

import numpy as np
import ml_dtypes

from concourse import bass, mybir, tile
from concourse.bass_utils import run_bass_kernel_spmd

N = 8192
F_IN = 512
F_OUT = 128
CORES = 8
B = N // CORES          # rows per core = 1024
T = N // 128            # j-tiles = 64
KT = F_IN // 128        # k-tiles for matmul1 = 4
MB = B // 128           # i-subtiles per core = 8
CW = 132                # padded h_aug width: [h(128) | s2(1) | pad(3)]

F32 = mybir.dt.float32
BF16 = mybir.dt.bfloat16
AF = mybir.ActivationFunctionType
ALU = mybir.AluOpType

_COMPILED = {}


def _split_multiwaits(nc):
    """This toolchain's walrus accepts at most ONE sync-wait command per
    instruction ("Too many sync wait commands" otherwise). Hoist extra waits
    onto standalone same-engine NoOps inserted right before the instruction —
    per-engine program order makes this semantically identical."""
    ctr = 0
    for f in nc.m.functions:
        for bb in f.blocks:
            insts = list(bb.instructions)
            if not any(
                ins.sync_info and ins.sync_info.on_wait and len(ins.sync_info.on_wait) > 1
                for ins in insts
            ):
                continue
            new = []
            for ins in insts:
                si = ins.sync_info
                if si and si.on_wait and len(si.on_wait) > 1:
                    waits = list(si.on_wait)
                    for w in waits[:-1]:
                        ctr += 1
                        nop = mybir.InstNoOp(name=f"WH-{ctr}", ins=[], outs=[])
                        nop.engine = ins.engine
                        nop.bass_nofuse = True
                        nop.sync_info = mybir.SyncInfo(on_wait=[w], on_update=[])
                        new.append(nop)
                    ins.sync_info = mybir.SyncInfo(
                        on_wait=[waits[-1]], on_update=list(si.on_update or [])
                    )
                new.append(ins)
            bb.instructions = new
    return nc


def build_nc():
    nc = bass.Bass()

    inT = nc.declare_dram_parameter("inputT", [F_IN, B], F32, isOutput=False)
    adjT = nc.declare_dram_parameter("adjT", [N, B], BF16, isOutput=False)
    waug = nc.declare_dram_parameter("Waug", [F_IN, CW], F32, isOutput=False)
    wa1 = nc.declare_dram_parameter("wa1", [F_IN, 1], F32, isOutput=False)
    out_e = nc.declare_dram_parameter("out", [B, F_OUT], F32, isOutput=True)

    cc_in = nc.dram_tensor("cc_in", [B, CW], F32)
    cc_out = nc.dram_tensor("cc_out", [N, CW], F32, addr_space="Shared")

    rg = [list(range(CORES))]

    with tile.TileContext(nc) as tc:
        with (
            tc.tile_pool(name="const", bufs=1) as cp,
            tc.tile_pool(name="adj", bufs=4) as ap,
            tc.tile_pool(name="act", bufs=3) as xp,
            tc.tile_pool(name="vec", bufs=3) as vp,
            tc.tile_pool(name="small", bufs=4) as sp,
            tc.tile_pool(name="epi", bufs=2) as ep,
        ):
            # ---- persistent SBUF tensors ----
            inT_sb = cp.tile([128, KT, B], F32, tag="inT")
            waug_sb = cp.tile([128, KT, CW], F32, tag="waug")
            wa1_sb = cp.tile([128, KT], F32, tag="wa1")
            h_all = cp.tile([128, T, CW], F32, tag="h_all")
            hb = cp.tile([128, T, CW], BF16, tag="hb")
            s1b = cp.tile([128, B], F32, tag="s1b")
            ones1 = cp.tile([1, 128], F32, tag="ones1")
            s1row = cp.tile([1, B], F32, tag="s1row")

            nc.sync.dma_start(out=inT_sb[:], in_=inT.rearrange("(k p) i -> p k i", p=128))
            nc.sync.dma_start(out=waug_sb[:], in_=waug.rearrange("(k p) c -> p k c", p=128))
            nc.sync.dma_start(out=wa1_sb[:], in_=wa1.rearrange("(k p) c -> p (k c)", p=128))
            nc.vector.memset(ones1[:], 1.0)

            # ---- phase 1: h_aug_local = inputT.T @ Waug, s1row = wa1.T @ inputT ----
            with tc.tile_pool(name="ps1", bufs=2, space="PSUM") as ps1:
                for m in range(MB):
                    pt = ps1.tile([128, CW], F32, tag="mm1")
                    for k in range(KT):
                        nc.tensor.matmul(
                            pt[:],
                            inT_sb[:, k, m * 128:(m + 1) * 128],
                            waug_sb[:, k, :],
                            start=(k == 0),
                            stop=(k == KT - 1),
                        )
                    hl = sp.tile([128, CW], F32, tag="hloc")
                    nc.vector.tensor_copy(hl[:], pt[:])
                    nc.sync.dma_start(out=cc_in[m * 128:(m + 1) * 128, :], in_=hl[:])

                # s1row in two 512-wide halves (f32 moving-operand limit)
                for hhalf in range(2):
                    pt = ps1.tile([1, 512], F32, tag="s1p")
                    sl = slice(hhalf * 512, (hhalf + 1) * 512)
                    for k in range(KT):
                        nc.tensor.matmul(
                            pt[:],
                            wa1_sb[:, k:k + 1],
                            inT_sb[:, k, sl],
                            start=(k == 0),
                            stop=(k == KT - 1),
                        )
                    nc.vector.tensor_copy(s1row[:, sl], pt[:])

                # broadcast s1row across partitions: S1B = ones1.T @ s1row
                for hhalf in range(2):
                    pt = ps1.tile([128, 512], F32, tag="s1bb")
                    sl = slice(hhalf * 512, (hhalf + 1) * 512)
                    nc.tensor.matmul(pt[:], ones1[:], s1row[:, sl], start=True, stop=True)
                    nc.vector.tensor_copy(s1b[:, sl], pt[:])

            # ---- phase 2: AllGather h_aug ----
            nc.gpsimd.collective_compute(
                "AllGather",
                ALU.bypass,
                replica_groups=rg,
                ins=[cc_in.ap().opt()],
                outs=[cc_out.ap().opt()],
            )
            nc.sync.dma_start(out=h_all[:], in_=cc_out.rearrange("(t p) c -> p t c", p=128))

            # ---- phase 3: bf16 rhs [h | 1] ----
            nc.vector.tensor_copy(hb[:, :, 0:128], h_all[:, :, 0:128])
            nc.vector.memset(hb[:, :, 128:129], 1.0)

            # ---- phase 4: main loop over j-tiles ----
            with tc.tile_pool(name="acc", bufs=1, space="PSUM") as accp:
                acc = [
                    accp.tile([128, CW], F32, tag=f"acc{c}", name=f"acc{c}")
                    for c in range(MB)
                ]

                for t in range(T):
                    at = ap.tile([128, B], BF16, tag="adjt")
                    nc.sync.dma_start(out=at[:], in_=adjT[t * 128:(t + 1) * 128, :])

                    s2 = h_all[:, t, 128:129]
                    s2s = sp.tile([128, 1], F32, tag="s2s")
                    nc.vector.tensor_scalar_mul(s2s[:], s2, 0.2)

                    xt = xp.tile([128, B], BF16, tag="xt")
                    nc.scalar.activation(xt[:], s1b[:], AF.Exp, bias=s2, scale=1.0)
                    yt = xp.tile([128, B], BF16, tag="yt")
                    nc.scalar.activation(yt[:], s1b[:], AF.Exp, bias=s2s[:], scale=0.2)

                    zt = vp.tile([128, B], BF16, tag="zt")
                    nc.vector.tensor_max(zt[:], xt[:], yt[:])
                    pt_ = vp.tile([128, B], BF16, tag="pt")
                    nc.vector.tensor_mul(pt_[:], zt[:], at[:])

                    for c in range(MB):
                        nc.tensor.matmul(
                            acc[c][:, 0:129],
                            pt_[:, c * 128:(c + 1) * 128],
                            hb[:, t, 0:129],
                            start=(t == 0),
                            stop=(t == T - 1),
                        )

                # ---- phase 5: epilogue: divide by rowsum, elu, store ----
                for c in range(MB):
                    rs = ep.tile([128, 1], F32, tag="rs")
                    nc.vector.reciprocal(rs[:], acc[c][:, 128:129])
                    hp = ep.tile([128, 128], F32, tag="hp")
                    nc.vector.tensor_scalar_mul(hp[:], acc[c][:, 0:128], rs[:])
                    mn = ep.tile([128, 128], F32, tag="mn")
                    nc.vector.tensor_scalar_min(mn[:], hp[:], 0.0)
                    ex = ep.tile([128, 128], F32, tag="ex")
                    nc.scalar.activation(ex[:], mn[:], AF.Exp, bias=0.0, scale=1.0)
                    nc.vector.tensor_scalar_add(ex[:], ex[:], -1.0)
                    oo = ep.tile([128, 128], F32, tag="oo")
                    nc.vector.tensor_max(oo[:], hp[:], ex[:])
                    nc.sync.dma_start(out=out_e[c * 128:(c + 1) * 128, :], in_=oo[:])

    return _split_multiwaits(nc)


def _prep_inputs(input, adj, W, a):
    input = np.asarray(input, dtype=np.float32)
    adj = np.asarray(adj, dtype=np.float32)
    W = np.asarray(W, dtype=np.float32)
    a = np.asarray(a, dtype=np.float32)

    wa1 = (W @ a[:F_OUT, 0]).astype(np.float32)          # [512]
    wa2 = (W @ a[F_OUT:, 0]).astype(np.float32)          # [512]
    waug = np.zeros((F_IN, CW), dtype=np.float32)
    waug[:, 0:F_OUT] = W
    waug[:, F_OUT] = wa2

    inputT = np.ascontiguousarray(input.T)               # [512, 8192]
    adjT = adj.T.astype(ml_dtypes.bfloat16)              # [8192, 8192] exact 0/1

    in_maps = []
    for r in range(CORES):
        sl = slice(r * B, (r + 1) * B)
        in_maps.append({
            "inputT": np.ascontiguousarray(inputT[:, sl]),
            "adjT": np.ascontiguousarray(adjT[:, sl]),
            "Waug": waug,
            "wa1": wa1.reshape(F_IN, 1),
        })
    return in_maps


def run(inputs, trace=False, trace_cores=None):
    """Returns (output [8192,128] f32, exec_time_ns or None)."""
    if "nc" not in _COMPILED:
        _COMPILED["nc"] = build_nc()
    nc = _COMPILED["nc"]
    in_maps = _prep_inputs(**inputs)
    res = run_bass_kernel_spmd(
        nc, in_maps, core_ids=list(range(CORES)),
        trace=trace, trace_cores=trace_cores,
    )
    out = np.concatenate([res.results[r]["out"] for r in range(CORES)], axis=0)
    return out.astype(np.float32), res.exec_time_ns


def kernel(**inputs):
    out, _ = run(inputs, trace=False)
    return out
